# revision 17
# baseline (speedup 1.0000x reference)
"""Trainium2 Bass kernel for LocalSingularityStrength (multi-scale box-filter
OLS slope + BN inference), data-parallel over 8 NeuronCores.

Math (exact restructure of the reference):
  reference: xs = (x - mn)/(mx - mn + eps); m_r = boxsum(xs);
             alpha = sum_r q_r ln(m_r + eps); out = alpha*A + B
  Since sum_r q_r == 0, the 1/(mx-mn+eps) factor cancels inside the log-slope:
             alpha = sum_r q_r ln( boxsum(x) - mn*cnt_r )
  where cnt_r(h,w) = #in-bounds cells of the SAME-padded window (separable:
  cnt_h(h)*cnt_w(w)).  So the normalize pass disappears and only min(x) per
  sample is needed.

Mapping per core (2 samples, H=224 -> 2 row-jobs of M=112 output rows):
  - cast x -> x~ = -x in fp16 (GpSimd), with a constant +1.0 row at partition
    120; the W-direction doubling cascade (VectorE, fp16 2x) then produces
    both the W-box-sums of -x and (from the 1.0 row) the cnt_w(w) patterns.
  - max-reduce of x~ gives mx~ = -mn (VectorE fp16 2x + one gpsimd
    partition_all_reduce per sample).
  - H-direction box sums: banded matmuls on TensorE; the band holds -1 in the
    window rows plus a runtime-written aug entry mx~*cnt_h(m) at partition
    120, so PSUM = boxsum(x) - mn*cnt exactly.  Scale-paired PSUM tiles
    (2,16) and (4,8), 4 banks each -> double buffered.
  - ln on ScalarE straight out of PSUM (fp32 -> fp16).
  - OLS combine: d0 = L16-L2, d1 = L8-L4 (VectorE fp16 2x), v = d1/3
    (GpSimd), u = d0 + v (VectorE), out = u*A + B in fp32 (GpSimd).
"""

import sys

sys.path.insert(0, "/opt/trn_rl_repo")

import numpy as np

import concourse.bacc as bacc
import concourse.bass as bass
import concourse.tile as tile
from concourse import mybir
from concourse.bass_utils import run_bass_kernel_spmd

FP16 = mybir.dt.float16
FP32 = mybir.dt.float32
ALU = mybir.AluOpType
ACT = mybir.ActivationFunctionType

NCORES = 8
SCALES = [2, 4, 8, 16]
NS = len(SCALES)
EPS_K = 1e-7
BN_EPS = 1e-3
PAD_L = 7
PAD_R = 9  # WP = W + 16 (one spare col on the right)
AUG_P = 120  # partition row holding the constant-1.0 / aug entries (both jobs)
K1 = AUG_P + 1  # contraction rows for every matmul

# OLS weights: alpha = q16*[(L16 - L2) + (1/3)(L8 - L4)]
_ls = np.log(np.array(SCALES, dtype=np.float64))
_dls = _ls - _ls.mean()
_den = float((_dls**2).sum())
Q16 = float(_dls[3] / _den)
C_RATIO = float(_dls[2] / _dls[3])  # exactly 1/3


def _jobs(H):
    """(a0, lo_in, K) for the two 112-row jobs of one 224-row sample."""
    M = H // 2
    out = []
    for a in (0, M):
        lo = max(0, a - 7)
        hi = min(H - 1, a + M - 1 + 8)
        out.append((a, lo, hi - lo + 1))
    return out


def _make_consts(H, W, C):
    """Static band / bandaug / padded-ones-row host arrays."""
    jobs = _jobs(H)
    M = H // 2
    WP = W + PAD_L + PAD_R
    bands = np.zeros((128, 2 * NS, M), np.float16)
    bandaug = np.zeros((128, 2 * NS, M), np.float16)
    for j, (a0, lo_in, K) in enumerate(jobs):
        for si, r in enumerate(SCALES):
            lo = (r - 1) // 2
            hi = r // 2
            for m in range(M):
                h = a0 + m
                r0 = max(0, h - lo)
                r1 = min(H - 1, h + hi)
                bands[r0 - lo_in : r1 - lo_in + 1, j * NS + si, m] = -1.0
                bandaug[AUG_P, j * NS + si, m] = float(r1 - r0 + 1)
    onesrow = np.zeros((1, WP * C), np.float16)
    onesrow[0, PAD_L * C : (PAD_L + W) * C] = 1.0
    return bands, bandaug, onesrow


# (uniform, A_imm, B_imm) — set by kernel() before build; default uniform
_BN_MODE = (True, Q16, 0.0)

_PROG_CACHE = {}


def build_program(BS, H, W, C, n_cores=NCORES):
    assert H == 224 and W % 2 == 0 and C == 32
    M = H // 2
    WP = W + PAD_L + PAD_R
    CW = 32  # w-columns per post-processing chunk (1024 elements)
    NCHUNK = W // CW
    HW_C = (W // 2) * C  # half-row elements for the split input DMA
    jobs = _jobs(H)

    uniform, a_imm, b_imm = _BN_MODE

    nc = bacc.Bacc("TRN2", target_bir_lowering=False, debug=False, num_devices=n_cores)
    x_in = nc.dram_tensor("x", [BS, H, W, C], FP32, kind="ExternalInput")
    bands_in = nc.dram_tensor("bands", [128, 2 * NS, M], FP16, kind="ExternalInput")
    bandaug_in = nc.dram_tensor("bandaug", [128, 2 * NS, M], FP16, kind="ExternalInput")
    ones_in = nc.dram_tensor("onesrow", [1, WP * C], FP16, kind="ExternalInput")
    scq_in = nc.dram_tensor("scq", [C], FP32, kind="ExternalInput")
    bi_in = nc.dram_tensor("bi", [C], FP32, kind="ExternalInput")
    out_t = nc.dram_tensor("out", [BS, H, W, C], FP32, kind="ExternalOutput")

    with tile.TileContext(nc) as tc:
        with (
            tc.tile_pool(name="consts", bufs=1) as consts,
            tc.tile_pool(name="xraw", bufs=2) as xraw_pool,
            tc.tile_pool(name="wide", bufs=1) as wide,
            tc.tile_pool(name="small", bufs=4) as small,
            tc.tile_pool(name="lca", bufs=2) as lca_pool,
            tc.tile_pool(name="lcb", bufs=2) as lcb_pool,
            tc.tile_pool(name="dd", bufs=2) as dd_pool,
            tc.tile_pool(name="outs", bufs=2) as outs,
            tc.tile_pool(name="psum", bufs=1, space="PSUM") as psum_pool,
        ):
            # ---- static constants ----
            eps_sb = consts.tile([128, 1], FP32)
            nc.vector.memset(eps_sb, 1e-6)
            band_sb = consts.tile([128, 2 * NS, M], FP16)
            nc.sync.dma_start(
                out=band_sb, in_=bands_in.rearrange("k s m -> k (s m)")
            )
            bandaug_sb = consts.tile([128, 2 * NS, M], FP16)
            nc.sync.dma_start(
                out=bandaug_sb, in_=bandaug_in.rearrange("k s m -> k (s m)")
            )
            # second static copy of the band rows; used to restore the data
            # rows inside the [96:121] aug-update window each sample
            bands2_sb = consts.tile([128, 2 * NS, M], FP16)
            nc.sync.dma_start(
                out=bands2_sb, in_=bands_in.rearrange("k s m -> k (s m)")
            )
            if not uniform:
                scq_sb = consts.tile([128, C], FP32)
                bi_sb = consts.tile([128, C], FP32)
                for dst, src in ((scq_sb, scq_in), (bi_sb, bi_in)):
                    nc.sync.dma_start(
                        out=dst,
                        in_=bass.AP(tensor=src.tensor, offset=0, ap=[[0, 128], [1, C]]),
                    )

            # ---- persistent working tiles ----
            xt = wide.tile([128, WP * C], FP16, name="xt", tag="xt")
            nc.vector.memset(xt[:, 0 : PAD_L * C], 0.0)
            nc.vector.memset(xt[:, (PAD_L + W) * C : WP * C], 0.0)
            # constant 1.0 row (data cols) at partition AUG_P; never overwritten
            nc.sync.dma_start(out=xt[AUG_P : AUG_P + 1, :], in_=ones_in[0:1, :])

            wt = [
                {
                    r: wide.tile([128, WP * C], FP16, name=f"w{j}_{r}", tag=f"w{j}_{r}")
                    for r in SCALES
                }
                for j in range(2)
            ]

            def shift_add(dst, src, w0, w1, d0, d1):
                # dst[w'] = src[w'+d0] + src[w'+d1] over w' in [w0, w1), rows 0..K1
                nc.vector.tensor_tensor(
                    out=dst[0:K1, w0 * C : w1 * C],
                    in0=src[0:K1, (w0 + d0) * C : (w1 + d0) * C],
                    in1=src[0:K1, (w0 + d1) * C : (w1 + d1) * C],
                    op=ALU.add,
                )

            for b in range(BS):
                rmax = []
                for j, (a0, lo_in, K) in enumerate(jobs):
                    # ---- load (half-width pieces, double buffered) + cast ----
                    for h in range(2):
                        xr = xraw_pool.tile([128, HW_C], FP32, tag="xr")
                        nc.sync.dma_start(
                            out=xr[0:K],
                            in_=x_in[
                                b, lo_in : lo_in + K, h * (W // 2) : (h + 1) * (W // 2)
                            ].rearrange("k w c -> k (w c)"),
                        )
                        nc.gpsimd.tensor_scalar(
                            out=xt[
                                0:K,
                                (PAD_L + h * (W // 2)) * C : (PAD_L + (h + 1) * (W // 2)) * C,
                            ],
                            in0=xr[0:K],
                            scalar1=-1.0,
                            scalar2=None,
                            op0=ALU.mult,
                        )
                    # ---- per-job row max of -x (fp16, 2x) ----
                    rm = small.tile([128, 1], FP32, tag=f"rmax{j}")
                    nc.vector.memset(rm, -1e30)
                    nc.vector.tensor_reduce(
                        out=rm[0:K],
                        in_=xt[0:K, PAD_L * C : (PAD_L + W) * C].rearrange(
                            "p (w c) -> p w c", c=C
                        ),
                        axis=mybir.AxisListType.XY,
                        op=ALU.max,
                    )
                    rmax.append(rm)
                    # ---- W-direction doubling cascade (rows 0..AUG_P incl.) ----
                    shift_add(wt[j][2], xt, 0, WP - 1, 0, 1)
                    shift_add(wt[j][4], wt[j][2], 1, WP - 2, -1, 1)
                    shift_add(wt[j][8], wt[j][4], 3, WP - 5, -2, 2)
                    shift_add(wt[j][16], wt[j][8], 7, PAD_L + W, -4, 4)

                # ---- sample max: mx~ = -mn, all-reduced to every partition ----
                cmb = small.tile([128, 1], FP32, tag="cmb")
                nc.vector.tensor_tensor(
                    out=cmb, in0=rmax[0], in1=rmax[1], op=ALU.max
                )
                mxb = small.tile([128, 1], FP32, tag="mxb")
                nc.gpsimd.partition_all_reduce(
                    mxb, cmb, channels=128, reduce_op=bass.bass_isa.ReduceOp.max
                )

                # ---- runtime band aug entries: mx~ * cnt_h at partition 120.
                # Engine ops must start at a multiple-of-32 partition, so update
                # rows [96:121]: bandaug is zero on the data rows there and the
                # static band is re-added via op1. ----
                for j in range(2):
                    nc.vector.scalar_tensor_tensor(
                        out=band_sb[96 : AUG_P + 1, j * NS : (j + 1) * NS, :],
                        in0=bandaug_sb[96 : AUG_P + 1, j * NS : (j + 1) * NS, :],
                        scalar=mxb[96 : AUG_P + 1],
                        in1=bands2_sb[96 : AUG_P + 1, j * NS : (j + 1) * NS, :],
                        op0=ALU.mult,
                        op1=ALU.add,
                    )

                # ---- per-chunk: matmuls + ln + OLS combine + BN ----
                for j, (a0, lo_in, K) in enumerate(jobs):
                    for ci in range(NCHUNK):
                        cw0 = (PAD_L + ci * CW) * C
                        # scale pair (2, 16) -> psA ; (4, 8) -> psB
                        psA = psum_pool.tile([M, 2, 1024], FP32)
                        psB = psum_pool.tile([M, 2, 1024], FP32)
                        for half in range(2):
                            o = half * 512
                            nc.tensor.matmul(
                                psA[:, 0, o : o + 512],
                                lhsT=band_sb[0:K1, j * NS + 0, :],
                                rhs=wt[j][2][0:K1, cw0 + o : cw0 + o + 512],
                                start=True,
                                stop=True,
                            )
                        for half in range(2):
                            o = half * 512
                            nc.tensor.matmul(
                                psA[:, 1, o : o + 512],
                                lhsT=band_sb[0:K1, j * NS + 3, :],
                                rhs=wt[j][16][0:K1, cw0 + o : cw0 + o + 512],
                                start=True,
                                stop=True,
                            )
                        lca = lca_pool.tile([M, 2, 1024], FP16, tag="lca")
                        nc.scalar.activation(
                            out=lca, in_=psA, func=ACT.Ln, bias=eps_sb[0:M], scale=1.0
                        )
                        for half in range(2):
                            o = half * 512
                            nc.tensor.matmul(
                                psB[:, 0, o : o + 512],
                                lhsT=band_sb[0:K1, j * NS + 1, :],
                                rhs=wt[j][4][0:K1, cw0 + o : cw0 + o + 512],
                                start=True,
                                stop=True,
                            )
                        for half in range(2):
                            o = half * 512
                            nc.tensor.matmul(
                                psB[:, 1, o : o + 512],
                                lhsT=band_sb[0:K1, j * NS + 2, :],
                                rhs=wt[j][8][0:K1, cw0 + o : cw0 + o + 512],
                                start=True,
                                stop=True,
                            )
                        lcb = lcb_pool.tile([M, 2, 1024], FP16, tag="lcb")
                        nc.scalar.activation(
                            out=lcb, in_=psB, func=ACT.Ln, bias=eps_sb[0:M], scale=1.0
                        )
                        # d0 = L16 - L2 ; d1 = L8 - L4   (fp16 2x)
                        d0 = dd_pool.tile([M, 1024], FP16, tag="d0")
                        nc.vector.tensor_tensor(
                            out=d0, in0=lca[:, 1, :], in1=lca[:, 0, :], op=ALU.subtract
                        )
                        d1 = dd_pool.tile([M, 1024], FP16, tag="d1")
                        nc.vector.tensor_tensor(
                            out=d1, in0=lcb[:, 1, :], in1=lcb[:, 0, :], op=ALU.subtract
                        )
                        # v = d1/3 (gpsimd), u = d0 + v (DVE 2x)
                        v = dd_pool.tile([M, 1024], FP16, tag="v")
                        nc.gpsimd.tensor_scalar(
                            out=v, in0=d1, scalar1=C_RATIO, scalar2=None, op0=ALU.mult
                        )
                        # u overwrites d1 (dead once v is computed)
                        u = d1
                        nc.vector.tensor_tensor(out=u, in0=d0, in1=v, op=ALU.add)
                        oc = outs.tile([M, 1024], FP32, tag="oc")
                        if uniform:
                            nc.gpsimd.tensor_scalar(
                                out=oc, in0=u, scalar1=a_imm, scalar2=b_imm,
                                op0=ALU.mult, op1=ALU.add,
                            )
                        else:
                            m1 = outs.tile([M, 1024], FP32, tag="m1")
                            scq_ap = bass.AP(
                                tensor=scq_sb.tensor, offset=scq_sb.offset,
                                ap=[scq_sb.ap[0][:], [0, CW], [1, C]],
                            )
                            bi_ap = bass.AP(
                                tensor=bi_sb.tensor, offset=bi_sb.offset,
                                ap=[bi_sb.ap[0][:], [0, CW], [1, C]],
                            )
                            nc.vector.tensor_tensor(
                                out=m1.rearrange("p (w c) -> p w c", c=C),
                                in0=u.rearrange("p (w c) -> p w c", c=C),
                                in1=scq_ap[0:M], op=ALU.mult,
                            )
                            nc.vector.tensor_tensor(
                                out=oc.rearrange("p (w c) -> p w c", c=C),
                                in0=m1.rearrange("p (w c) -> p w c", c=C),
                                in1=bi_ap[0:M], op=ALU.add,
                            )
                        nc.sync.dma_start(
                            out=out_t[b, a0 : a0 + M, ci * CW : (ci + 1) * CW, :].rearrange(
                                "m w c -> m (w c)"
                            ),
                            in_=oc,
                        )

    nc.compile()
    return nc


def _get_program(BS, H, W, C, bn_mode):
    key = (BS, H, W, C, bn_mode)
    if key not in _PROG_CACHE:
        global _BN_MODE
        _BN_MODE = bn_mode
        _PROG_CACHE[key] = build_program(BS, H, W, C)
    return _PROG_CACHE[key]


def make_in_maps(x_np, gamma, beta, moving_mean, moving_var):
    """Shard + host-fold BN; returns (in_maps, bn_mode)."""
    B, H, W, C = x_np.shape
    BS = B // NCORES
    sc = gamma / np.sqrt(moving_var + np.float32(BN_EPS))
    scq = (sc * np.float32(Q16)).astype(np.float32)
    bi = (beta - moving_mean * sc).astype(np.float32)
    uniform = bool(np.ptp(scq) == 0 and np.ptp(bi) == 0)
    bn_mode = (uniform, float(scq[0]), float(bi[0])) if uniform else (False, 0.0, 0.0)
    bands, bandaug, onesrow = _make_consts(H, W, C)
    in_maps = []
    for i in range(NCORES):
        in_maps.append(
            {
                "x": x_np[i * BS : (i + 1) * BS],
                "bands": bands,
                "bandaug": bandaug,
                "onesrow": onesrow,
                "scq": scq,
                "bi": bi,
            }
        )
    return in_maps, bn_mode


def kernel(x, gamma, beta, moving_mean, moving_var):
    x = np.ascontiguousarray(np.asarray(x), dtype=np.float32)
    gamma = np.asarray(gamma, dtype=np.float32)
    beta = np.asarray(beta, dtype=np.float32)
    moving_mean = np.asarray(moving_mean, dtype=np.float32)
    moving_var = np.asarray(moving_var, dtype=np.float32)

    B, H, W, C = x.shape
    assert B % NCORES == 0
    BS = B // NCORES

    in_maps, bn_mode = make_in_maps(x, gamma, beta, moving_mean, moving_var)
    nc = _get_program(BS, H, W, C, bn_mode)
    res = run_bass_kernel_spmd(nc, in_maps, list(range(NCORES)))
    out = np.concatenate([res.results[i]["out"] for i in range(NCORES)], axis=0)
    return out.astype(np.float32)


# revision 21
# speedup vs baseline: 4.0699x; 4.0699x over previous
"""Trainium2 Bass kernel for LocalSingularityStrength (multi-scale box-filter
OLS slope + BN inference), data-parallel over 8 NeuronCores.

Math (exact restructure of the reference):
  reference: xs = (x - mn)/(mx - mn + eps); m_r = boxsum(xs);
             alpha = sum_r q_r ln(m_r + eps); out = alpha*A + B
  Since sum_r q_r == 0, the 1/(mx-mn+eps) factor cancels inside the log-slope:
             alpha = sum_r q_r ln( boxsum(x) - mn*cnt_r )
  where cnt_r(h,w) = #in-bounds cells of the SAME-padded window (separable:
  cnt_h(h)*cnt_w(w)).  So the normalize pass disappears and only min(x) per
  sample is needed.

Mapping per core (2 samples, H=224 -> 2 row-jobs of M=112 output rows):
  - cast x -> x~ = -x in fp16 (GpSimd), with a constant +1.0 row at partition
    120; the W-direction doubling cascade (VectorE, fp16 2x) then produces
    both the W-box-sums of -x and (from the 1.0 row) the cnt_w(w) patterns.
  - max-reduce of x~ gives mx~ = -mn (VectorE fp16 2x + one gpsimd
    partition_all_reduce per sample).
  - H-direction box sums: banded matmuls on TensorE; the band holds -1 in the
    window rows plus a runtime-written aug entry mx~*cnt_h(m) at partition
    120, so PSUM = boxsum(x) - mn*cnt exactly.  Scale-paired PSUM tiles
    (2,16) and (4,8), 4 banks each -> double buffered.
  - ln on ScalarE straight out of PSUM (fp32 -> fp16).
  - OLS combine: d0 = L16-L2, d1 = L8-L4 (VectorE fp16 2x), v = d1/3
    (GpSimd), u = d0 + v (VectorE), out = u*A + B in fp32 (GpSimd).
"""

import sys

sys.path.insert(0, "/opt/trn_rl_repo")

import numpy as np

import concourse.bacc as bacc
import concourse.bass as bass
import concourse.tile as tile
from concourse import mybir
from concourse.bass_utils import run_bass_kernel_spmd

FP16 = mybir.dt.float16
FP32 = mybir.dt.float32
ALU = mybir.AluOpType
ACT = mybir.ActivationFunctionType

NCORES = 8
SCALES = [2, 4, 8, 16]
NS = len(SCALES)
EPS_K = 1e-7
BN_EPS = 1e-3
PAD_L = 7
PAD_R = 9  # WP = W + 16 (one spare col on the right)
AUG_P = 120  # partition row holding the constant-1.0 / aug entries (both jobs)
K1 = AUG_P + 1  # contraction rows for every matmul

# OLS weights: alpha = q16*[(L16 - L2) + (1/3)(L8 - L4)]
_ls = np.log(np.array(SCALES, dtype=np.float64))
_dls = _ls - _ls.mean()
_den = float((_dls**2).sum())
Q16 = float(_dls[3] / _den)
C_RATIO = float(_dls[2] / _dls[3])  # exactly 1/3


def _jobs(H):
    """(a0, lo_in, K) for the two 112-row jobs of one 224-row sample."""
    M = H // 2
    out = []
    for a in (0, M):
        lo = max(0, a - 7)
        hi = min(H - 1, a + M - 1 + 8)
        out.append((a, lo, hi - lo + 1))
    return out


def _make_consts(H, W, C):
    """Static band / bandaug / padded-ones-row host arrays."""
    jobs = _jobs(H)
    M = H // 2
    WP = W + PAD_L + PAD_R
    bands = np.zeros((128, 2 * NS, M), np.float16)
    bandaug = np.zeros((128, 2 * NS, M), np.float16)
    for j, (a0, lo_in, K) in enumerate(jobs):
        for si, r in enumerate(SCALES):
            lo = (r - 1) // 2
            hi = r // 2
            for m in range(M):
                h = a0 + m
                r0 = max(0, h - lo)
                r1 = min(H - 1, h + hi)
                bands[r0 - lo_in : r1 - lo_in + 1, j * NS + si, m] = -1.0
                bandaug[AUG_P, j * NS + si, m] = float(r1 - r0 + 1)
    onesrow = np.zeros((1, WP * C), np.float16)
    onesrow[0, PAD_L * C : (PAD_L + W) * C] = 1.0
    return bands, bandaug, onesrow


# (uniform, A_imm, B_imm) — set by kernel() before build; default uniform
_BN_MODE = (True, Q16, 0.0)

_PROG_CACHE = {}


def build_program(BS, H, W, C, n_cores=NCORES):
    assert H == 224 and W % 2 == 0 and C == 32
    M = H // 2
    WP = W + PAD_L + PAD_R
    CW = 32  # w-columns per post-processing chunk (1024 elements)
    NCHUNK = W // CW
    HW_C = (W // 2) * C  # half-row elements for the split input DMA
    jobs = _jobs(H)

    uniform, a_imm, b_imm = _BN_MODE

    nc = bacc.Bacc("TRN2", target_bir_lowering=False, debug=False, num_devices=n_cores)
    x_in = nc.dram_tensor("x", [BS, H, W, C], FP32, kind="ExternalInput")
    bands_in = nc.dram_tensor("bands", [128, 2 * NS, M], FP16, kind="ExternalInput")
    bandaug_in = nc.dram_tensor("bandaug", [128, 2 * NS, M], FP16, kind="ExternalInput")
    ones_in = nc.dram_tensor("onesrow", [1, WP * C], FP16, kind="ExternalInput")
    scq_in = nc.dram_tensor("scq", [C], FP32, kind="ExternalInput")
    bi_in = nc.dram_tensor("bi", [C], FP32, kind="ExternalInput")
    out_t = nc.dram_tensor("out", [BS, H, W, C], FP32, kind="ExternalOutput")

    with tile.TileContext(nc) as tc:
        with (
            tc.tile_pool(name="consts", bufs=1) as consts,
            tc.tile_pool(name="xraw", bufs=2) as xraw_pool,
            tc.tile_pool(name="wide", bufs=1) as wide,
            tc.tile_pool(name="small", bufs=4) as small,
            tc.tile_pool(name="lca", bufs=2) as lca_pool,
            tc.tile_pool(name="lcb", bufs=2) as lcb_pool,
            tc.tile_pool(name="dd", bufs=2) as dd_pool,
            tc.tile_pool(name="outs", bufs=2) as outs,
            tc.tile_pool(name="psum", bufs=1, space="PSUM") as psum_pool,
        ):
            # ---- static constants ----
            eps_sb = consts.tile([128, 1], FP32)
            nc.vector.memset(eps_sb, 1e-6)
            band_sb = consts.tile([128, 2 * NS, M], FP16)
            nc.sync.dma_start(
                out=band_sb, in_=bands_in.rearrange("k s m -> k (s m)")
            )
            bandaug_sb = consts.tile([128, 2 * NS, M], FP16)
            nc.sync.dma_start(
                out=bandaug_sb, in_=bandaug_in.rearrange("k s m -> k (s m)")
            )
            # second static copy of the band rows; used to restore the data
            # rows inside the [96:121] aug-update window each sample
            bands2_sb = consts.tile([128, 2 * NS, M], FP16)
            nc.sync.dma_start(
                out=bands2_sb, in_=bands_in.rearrange("k s m -> k (s m)")
            )
            if not uniform:
                scq_sb = consts.tile([128, C], FP32)
                bi_sb = consts.tile([128, C], FP32)
                for dst, src in ((scq_sb, scq_in), (bi_sb, bi_in)):
                    nc.sync.dma_start(
                        out=dst,
                        in_=bass.AP(tensor=src.tensor, offset=0, ap=[[0, 128], [1, C]]),
                    )

            # ---- persistent working tiles ----
            xt = wide.tile([128, WP * C], FP16, name="xt", tag="xt")
            nc.vector.memset(xt[:, 0 : PAD_L * C], 0.0)
            nc.vector.memset(xt[:, (PAD_L + W) * C : WP * C], 0.0)
            # constant 1.0 row (data cols) at partition AUG_P; never overwritten
            nc.sync.dma_start(out=xt[AUG_P : AUG_P + 1, :], in_=ones_in[0:1, :])

            wt = [
                {
                    r: wide.tile([128, WP * C], FP16, name=f"w{j}_{r}", tag=f"w{j}_{r}")
                    for r in SCALES
                }
                for j in range(2)
            ]

            def shift_add(dst, src, w0, w1, d0, d1):
                # dst[w'] = src[w'+d0] + src[w'+d1] over w' in [w0, w1), rows 0..K1
                nc.vector.tensor_tensor(
                    out=dst[0:K1, w0 * C : w1 * C],
                    in0=src[0:K1, (w0 + d0) * C : (w1 + d0) * C],
                    in1=src[0:K1, (w0 + d1) * C : (w1 + d1) * C],
                    op=ALU.add,
                )

            for b in range(BS):
                rmax = []
                for j, (a0, lo_in, K) in enumerate(jobs):
                    # ---- load (half-width pieces, double buffered) + cast ----
                    for h in range(2):
                        xr = xraw_pool.tile([128, HW_C], FP32, tag="xr")
                        nc.sync.dma_start(
                            out=xr[0:K],
                            in_=x_in[
                                b, lo_in : lo_in + K, h * (W // 2) : (h + 1) * (W // 2)
                            ].rearrange("k w c -> k (w c)"),
                        )
                        nc.scalar.activation(
                            out=xt[
                                0:K,
                                (PAD_L + h * (W // 2)) * C : (PAD_L + (h + 1) * (W // 2)) * C,
                            ],
                            in_=xr[0:K],
                            func=ACT.Copy,
                            bias=0.0,
                            scale=-1.0,
                        )
                    # ---- per-job row max of -x (DVE; flat X reduce, fp16 2x) ----
                    rm = small.tile([128, 1], FP32, tag=f"rmax{j}")
                    nc.vector.memset(rm, -1e30)
                    nc.vector.tensor_reduce(
                        out=rm[0:K],
                        in_=xt[0:K, PAD_L * C : (PAD_L + W) * C],
                        axis=mybir.AxisListType.X,
                        op=ALU.max,
                    )
                    rmax.append(rm)
                    # ---- W-direction doubling cascade (rows 0..AUG_P incl.) ----
                    shift_add(wt[j][2], xt, 0, WP - 1, 0, 1)
                    shift_add(wt[j][4], wt[j][2], 1, WP - 2, -1, 1)
                    shift_add(wt[j][8], wt[j][4], 3, WP - 5, -2, 2)
                    shift_add(wt[j][16], wt[j][8], 7, PAD_L + W, -4, 4)

                # ---- sample max: mx~ = -mn, all-reduced to every partition ----
                cmb = small.tile([128, 1], FP32, tag="cmb")
                nc.vector.tensor_tensor(
                    out=cmb, in0=rmax[0], in1=rmax[1], op=ALU.max
                )
                mxb = small.tile([128, 1], FP32, tag="mxb")
                nc.gpsimd.partition_all_reduce(
                    mxb, cmb, channels=128, reduce_op=bass.bass_isa.ReduceOp.max
                )

                # ---- runtime band aug entries: mx~ * cnt_h at partition 120.
                # Engine ops must start at a multiple-of-32 partition, so update
                # rows [96:121]: bandaug is zero on the data rows there and the
                # static band is re-added via op1. ----
                for j in range(2):
                    nc.vector.scalar_tensor_tensor(
                        out=band_sb[96 : AUG_P + 1, j * NS : (j + 1) * NS, :],
                        in0=bandaug_sb[96 : AUG_P + 1, j * NS : (j + 1) * NS, :],
                        scalar=mxb[96 : AUG_P + 1],
                        in1=bands2_sb[96 : AUG_P + 1, j * NS : (j + 1) * NS, :],
                        op0=ALU.mult,
                        op1=ALU.add,
                    )

                # ---- per-chunk: matmuls + ln + OLS combine + BN ----
                for j, (a0, lo_in, K) in enumerate(jobs):
                    for ci in range(NCHUNK):
                        cw0 = (PAD_L + ci * CW) * C
                        # scale pair (2, 16) -> psA ; (4, 8) -> psB
                        psA = psum_pool.tile([M, 2, 1024], FP32)
                        psB = psum_pool.tile([M, 2, 1024], FP32)
                        for half in range(2):
                            o = half * 512
                            nc.tensor.matmul(
                                psA[:, 0, o : o + 512],
                                lhsT=band_sb[0:K1, j * NS + 0, :],
                                rhs=wt[j][2][0:K1, cw0 + o : cw0 + o + 512],
                                start=True,
                                stop=True,
                            )
                        for half in range(2):
                            o = half * 512
                            nc.tensor.matmul(
                                psA[:, 1, o : o + 512],
                                lhsT=band_sb[0:K1, j * NS + 3, :],
                                rhs=wt[j][16][0:K1, cw0 + o : cw0 + o + 512],
                                start=True,
                                stop=True,
                            )
                        lca = lca_pool.tile([M, 2, 1024], FP16, tag="lca")
                        nc.scalar.activation(
                            out=lca, in_=psA, func=ACT.Ln, bias=eps_sb[0:M], scale=1.0
                        )
                        for half in range(2):
                            o = half * 512
                            nc.tensor.matmul(
                                psB[:, 0, o : o + 512],
                                lhsT=band_sb[0:K1, j * NS + 1, :],
                                rhs=wt[j][4][0:K1, cw0 + o : cw0 + o + 512],
                                start=True,
                                stop=True,
                            )
                        for half in range(2):
                            o = half * 512
                            nc.tensor.matmul(
                                psB[:, 1, o : o + 512],
                                lhsT=band_sb[0:K1, j * NS + 2, :],
                                rhs=wt[j][8][0:K1, cw0 + o : cw0 + o + 512],
                                start=True,
                                stop=True,
                            )
                        lcb = lcb_pool.tile([M, 2, 1024], FP16, tag="lcb")
                        nc.scalar.activation(
                            out=lcb, in_=psB, func=ACT.Ln, bias=eps_sb[0:M], scale=1.0
                        )
                        # d0 = L16 - L2 ; d1 = L8 - L4   (fp16 2x)
                        d0 = dd_pool.tile([M, 1024], FP16, tag="d0")
                        nc.vector.tensor_tensor(
                            out=d0, in0=lca[:, 1, :], in1=lca[:, 0, :], op=ALU.subtract
                        )
                        d1 = dd_pool.tile([M, 1024], FP16, tag="d1")
                        nc.vector.tensor_tensor(
                            out=d1, in0=lcb[:, 1, :], in1=lcb[:, 0, :], op=ALU.subtract
                        )
                        oc = outs.tile([M, 1024], FP32, tag="oc")
                        if uniform:
                            # v = d1*(A*C) + B on gpsimd (fp16->fp16, cheap);
                            # oc = d0*A + v on DVE (one STT, fp32 out)
                            v = dd_pool.tile([M, 1024], FP16, tag="v")
                            nc.gpsimd.tensor_scalar(
                                out=v, in0=d1,
                                scalar1=a_imm * C_RATIO, scalar2=b_imm,
                                op0=ALU.mult, op1=ALU.add,
                            )
                            nc.vector.scalar_tensor_tensor(
                                out=oc, in0=d0, scalar=a_imm, in1=v,
                                op0=ALU.mult, op1=ALU.add,
                            )
                        else:
                            v = dd_pool.tile([M, 1024], FP16, tag="v")
                            nc.gpsimd.tensor_scalar(
                                out=v, in0=d1, scalar1=C_RATIO, scalar2=None,
                                op0=ALU.mult,
                            )
                            u = d1
                            nc.vector.tensor_tensor(out=u, in0=d0, in1=v, op=ALU.add)
                            m1 = outs.tile([M, 1024], FP32, tag="m1")
                            scq_ap = bass.AP(
                                tensor=scq_sb.tensor, offset=scq_sb.offset,
                                ap=[scq_sb.ap[0][:], [0, CW], [1, C]],
                            )
                            bi_ap = bass.AP(
                                tensor=bi_sb.tensor, offset=bi_sb.offset,
                                ap=[bi_sb.ap[0][:], [0, CW], [1, C]],
                            )
                            nc.vector.tensor_tensor(
                                out=m1.rearrange("p (w c) -> p w c", c=C),
                                in0=u.rearrange("p (w c) -> p w c", c=C),
                                in1=scq_ap[0:M], op=ALU.mult,
                            )
                            nc.vector.tensor_tensor(
                                out=oc.rearrange("p (w c) -> p w c", c=C),
                                in0=m1.rearrange("p (w c) -> p w c", c=C),
                                in1=bi_ap[0:M], op=ALU.add,
                            )
                        nc.sync.dma_start(
                            out=out_t[b, a0 : a0 + M, ci * CW : (ci + 1) * CW, :].rearrange(
                                "m w c -> m (w c)"
                            ),
                            in_=oc,
                        )

    nc.compile()
    return nc


def _get_program(BS, H, W, C, bn_mode):
    key = (BS, H, W, C, bn_mode)
    if key not in _PROG_CACHE:
        global _BN_MODE
        _BN_MODE = bn_mode
        _PROG_CACHE[key] = build_program(BS, H, W, C)
    return _PROG_CACHE[key]


def make_in_maps(x_np, gamma, beta, moving_mean, moving_var):
    """Shard + host-fold BN; returns (in_maps, bn_mode)."""
    B, H, W, C = x_np.shape
    BS = B // NCORES
    sc = gamma / np.sqrt(moving_var + np.float32(BN_EPS))
    scq = (sc * np.float32(Q16)).astype(np.float32)
    bi = (beta - moving_mean * sc).astype(np.float32)
    uniform = bool(np.ptp(scq) == 0 and np.ptp(bi) == 0)
    bn_mode = (uniform, float(scq[0]), float(bi[0])) if uniform else (False, 0.0, 0.0)
    bands, bandaug, onesrow = _make_consts(H, W, C)
    in_maps = []
    for i in range(NCORES):
        in_maps.append(
            {
                "x": x_np[i * BS : (i + 1) * BS],
                "bands": bands,
                "bandaug": bandaug,
                "onesrow": onesrow,
                "scq": scq,
                "bi": bi,
            }
        )
    return in_maps, bn_mode


def kernel(x, gamma, beta, moving_mean, moving_var):
    x = np.ascontiguousarray(np.asarray(x), dtype=np.float32)
    gamma = np.asarray(gamma, dtype=np.float32)
    beta = np.asarray(beta, dtype=np.float32)
    moving_mean = np.asarray(moving_mean, dtype=np.float32)
    moving_var = np.asarray(moving_var, dtype=np.float32)

    B, H, W, C = x.shape
    assert B % NCORES == 0
    BS = B // NCORES

    in_maps, bn_mode = make_in_maps(x, gamma, beta, moving_mean, moving_var)
    nc = _get_program(BS, H, W, C, bn_mode)
    res = run_bass_kernel_spmd(nc, in_maps, list(range(NCORES)))
    out = np.concatenate([res.results[i]["out"] for i in range(NCORES)], axis=0)
    return out.astype(np.float32)
